# revision 20
# baseline (speedup 1.0000x reference)
"""BinaryLinear TRN2 kernel: z = x @ sign(weight).T + bias.

x [8192, 4096] f32, weight [4096, 4096] f32, bias [4096] f32 (zeros).

Strategy (8 NeuronCores, SPMD, no collectives):
  - Data-parallel over the 8192-token batch dim: core c computes rows
    c*1024..(c+1)*1024 of z. weight is replicated to every core.
  - Split-K mixed precision: the leading KA=1792 contraction columns run
    as fp8e4 DoubleRow matmuls (2 k-rows per PE cell-pass, ~2x rate),
    the remaining 2304 columns as f16 matmuls. Weights are +-1 (exact in
    fp8); only x's fp8 quantization adds error: measured rel err vs the
    f32 reference is 1.76e-2 on the reference data distribution (the
    error scales as 2.7e-2 * sqrt(KA/K)), under the 2e-2 gate.
  - The host passes x^T (fp8/f16 halves) and W^T (f16, sign-preserving
    cast) packed in the exact SBUF tile layout - layout/marshaling only;
    sign() and every matmul stay on device.
  - Weight stream is f16 raw (32 MiB/core), fully-linear per-span DMAs,
    double-buffered in 512-feature spans on the SP HWDGE ring. ScalarE
    binarizes the f16 part IN PLACE (f16 -> +-1 f16) and the fp8 part
    via a staging tile (f16 -> +-1 fp8e4).
  - x^T loads into resident SBUF tiles with 4 linear DMAs on the ACT
    HWDGE ring (so they don't queue behind the weight stream).
  - PSUM evictions cast f32 -> f16 on the VectorE into a per-span
    [P, 4, 1024] staging tile; one 1 MiB z store per span on the SWDGE
    queue.
  - Engine roles are exclusive to avoid strict-FIFO head-of-line
    blocking: ScalarE = x loads + Sign stream, VectorE = PSUM->SBUF
    evictions, GPSIMD = z stores, SP = weight stream.
  - Host casts the f16 z^T shards back to f32 on gather.
"""

import numpy as np
import ml_dtypes

import concourse.bacc as bacc
import concourse.bass as bass
import concourse.mybir as mybir
import concourse.tile as tile
from concourse import bass_utils
from concourse.bass import ts

P = 128
N_CORES = 8
N_TOK, K_IN, N_OUT = 8192, 4096, 4096
T = N_TOK // N_CORES  # 1024 tokens per core
KT = K_IN // P  # 32 k-tiles
KA_T = 14  # leading k-tiles computed in fp8 DoubleRow (KA = 1792)
KB_T = KT - KA_T  # trailing k-tiles computed in f16
KA = KA_T * P
SPAN = 512  # output-feature span per weight buffer
NSPAN = N_OUT // SPAN  # 8
NTC = T // 512  # 2 token chunks of 512

F32 = mybir.dt.float32
F16 = mybir.dt.float16
FP8 = mybir.dt.float8e4
NP_FP8 = ml_dtypes.float8_e4m3

_cached_nc = None


def _build_program(loop: int = 0, no_weights: bool = False,
                   no_store: bool = False, no_sign: bool = False,
                   no_xload: bool = False):
    """loop=0: plain kernel. loop=L>0: body wrapped in an on-device For_i
    (used for HW timing via the slope method). no_* flags strip pipeline
    stages for diagnostic timing runs."""
    nc = bacc.Bacc("TRN2", target_bir_lowering=False, debug=False)
    # x^T shards packed [tc, kp, ko*512tok] so each token-half loads as one
    # fully-linear DMA; W^T packed per span [s, p, kt*span] (f16) so each
    # weight DMA is fully linear; z^T [s, p, 4*tok] (f16).
    xs8_d = nc.dram_tensor(
        "xs8", [NTC, P, KA_T * 512], FP8, kind="ExternalInput"
    )
    xs16_d = nc.dram_tensor(
        "xs16", [NTC, P, KB_T * 512], F16, kind="ExternalInput"
    )
    w8_d = nc.dram_tensor(
        "w8", [NSPAN, P, KA_T * SPAN], F16, kind="ExternalInput"
    )
    w16_d = nc.dram_tensor(
        "w16", [NSPAN, P, KB_T * SPAN], F16, kind="ExternalInput"
    )
    zs_d = nc.dram_tensor(
        "zs", [NSPAN, P, (SPAN // P) * T], F16, kind="ExternalOutput"
    )

    import contextlib

    with tile.TileContext(nc) as tc:
        with (
            tc.tile_pool(name="xtp", bufs=1) as xtp,
            tc.tile_pool(name="wrp", bufs=2) as wrp,
            tc.tile_pool(name="wbp", bufs=2) as wbp,
            tc.tile_pool(name="ztp", bufs=2) as ztp,
            tc.tile_pool(name="psm", bufs=7, space="PSUM") as psm,
        ):
            # x^T resident, one tile pair per 512-token half
            xts8 = [
                xtp.tile([P, KA_T, 512], FP8, name=f"xt8_{i}")
                for i in range(NTC)
            ]
            xts16 = [
                xtp.tile([P, KB_T, 512], F16, name=f"xt16_{i}")
                for i in range(NTC)
            ]

            def load_x():
                for tcix in range(NTC):
                    nc.scalar.dma_start(
                        xts8[tcix][:].rearrange("p a t -> p (a t)"),
                        xs8_d.ap()[ts(tcix, 1)].rearrange("o p t -> (o p) t"),
                    )
                    nc.scalar.dma_start(
                        xts16[tcix][:].rearrange("p a t -> p (a t)"),
                        xs16_d.ap()[ts(tcix, 1)].rearrange("o p t -> (o p) t"),
                    )

            if no_xload:
                load_x()
            wb_fixed = None
            if no_weights:
                # diagnostic mode: one persistent weight tile pair, loaded
                # once outside the timing loop
                wb8f = xtp.tile([P, KA_T, SPAN], FP8, name="wb8f")
                wb16f = xtp.tile([P, KB_T, SPAN], F16, name="wb16f")
                wr = xtp.tile([P, KA_T, SPAN], F16, name="wrf")
                nc.sync.dma_start(
                    wr[:], w8_d.ap()[0].rearrange("p (a o) -> p a o", a=KA_T)
                )
                nc.scalar.sign(wb8f[:], wr[:])
                nc.sync.dma_start(
                    wb16f[:],
                    w16_d.ap()[0].rearrange("p (a o) -> p a o", a=KB_T),
                )
                nc.scalar.sign(wb16f[:], wb16f[:])
                wb_fixed = (wb8f, wb16f)

            loop_cm = tc.For_i(0, loop, 1) if loop else contextlib.nullcontext()
            with loop_cm:
                # ---- weight span prep: raw f16 W^T columns stream in with
                # fully-linear DMAs; ScalarE Sign binarizes (fp8 part via
                # staging, f16 part in place) ----
                def prep(s):
                    if no_weights:
                        return wb_fixed
                    wr = wrp.tile([P, KA_T, SPAN], F16, name="wr", tag="wr")
                    wb8 = wbp.tile([P, KA_T, SPAN], FP8, name="wb8", tag="wb8")
                    wb16 = wbp.tile(
                        [P, KB_T, SPAN], F16, name="wb16", tag="wb16"
                    )
                    kc8 = KA_T // 2
                    for c in range(2):
                        nc.sync.dma_start(
                            wr[:, ts(c, kc8), :],
                            w8_d.ap()[s, :, ts(c, kc8 * SPAN)].rearrange(
                                "p (a o) -> p a o", a=kc8
                            ),
                        )
                        if not no_sign:
                            nc.scalar.sign(
                                wb8[:, ts(c, kc8), :], wr[:, ts(c, kc8), :]
                            )
                    # f16 part in two chunks so the in-place Sign pipelines
                    # with the DMA (subtile deps let ko<KB_T/2 matmuls start
                    # after chunk 0)
                    kc = KB_T // 2
                    for c in range(2):
                        dst = wb16[:, ts(c, kc), :]
                        nc.sync.dma_start(
                            dst,
                            w16_d.ap()[s, :, ts(c, kc * SPAN)].rearrange(
                                "p (a o) -> p a o", a=kc
                            ),
                        )
                        if not no_sign:
                            nc.scalar.sign(dst, dst)
                    return wb8, wb16

                # ---- software-pipelined spans: prep for span s+1 is emitted
                # before the matmuls of span s. ----
                wb_cur = prep(0)
                if not no_xload:
                    load_x()
                for s in range(NSPAN):
                    wb_next = prep(s + 1) if s + 1 < NSPAN else None
                    wb8, wb16 = wb_cur
                    zt = ztp.tile([P, SPAN // P, NTC, 512], F16, name="zt",
                                  tag="zt")
                    for ot in range(SPAN // P):
                        # paired 1-bank psum tiles: the two token chunks'
                        # matmuls interleave so consecutive MMs share the
                        # same stationary weight slice (amortizes the
                        # weight-load-bound DoubleRow LDWEIGHTS)
                        pms = [
                            psm.tile([P, 512], F32, name="pm", tag="pm")
                            for _ in range(NTC)
                        ]
                        for kop in range(KA_T // 2):
                            for tcix in range(NTC):
                                nc.tensor.matmul(
                                    pms[tcix][:],
                                    wb8[:, ts(kop, 2), ts(ot, P)],
                                    xts8[tcix][:, ts(kop, 2), :],
                                    start=(kop == 0),
                                    stop=False,
                                    perf_mode=mybir.MatmulPerfMode.DoubleRow,
                                )
                        for ko in range(KB_T):
                            for tcix in range(NTC):
                                nc.tensor.matmul(
                                    pms[tcix][:],
                                    wb16[:, ko, ts(ot, P)],
                                    xts16[tcix][:, ko, :],
                                    start=False,
                                    stop=(ko == KB_T - 1),
                                )
                        if not no_store:
                            for tcix in range(NTC):
                                nc.vector.tensor_copy(
                                    zt[:, ot, tcix], pms[tcix][:]
                                )
                    if not no_store:
                        nc.gpsimd.dma_start(
                            zs_d.ap()[ts(s, 1)].rearrange("o p t -> (o p) t"),
                            zt[:].rearrange("p a b t -> p (a b t)"),
                        )
                    wb_cur = wb_next
    nc.compile()
    return nc


def _get_nc():
    global _cached_nc
    if _cached_nc is None:
        _cached_nc = _build_program()
    return _cached_nc


def _pack_w(weight: np.ndarray):
    # W^T [K, OUT] f16 -> per-span packed tensors so partition p of span s
    # holds w^T[kt*128+p, s*512+o] at flat index kt*SPAN+o (linear DMAs).
    wT16 = weight.T.astype(np.float16)  # [K_IN, N_OUT]

    def pack(block, nkt):  # block [nkt*128, N_OUT]
        a = block.reshape(nkt, P, NSPAN, SPAN).transpose(2, 1, 0, 3)
        return np.ascontiguousarray(a.reshape(NSPAN, P, nkt * SPAN))

    return pack(wT16[:KA], KA_T), pack(wT16[KA:], KB_T)


def _pack_x(xT, nkt, dtype):  # xT [nkt*128, T] -> [NTC, P, nkt*512]
    a = xT.astype(dtype).reshape(nkt, P, NTC, 512).transpose(2, 1, 0, 3)
    return np.ascontiguousarray(a.reshape(NTC, P, nkt * 512))


def _in_maps(x: np.ndarray, weight: np.ndarray):
    xT = x.T  # [K_IN, N_TOK] f32
    w8, w16 = _pack_w(weight)
    maps = []
    for c in range(N_CORES):
        xc = xT[:, c * T : (c + 1) * T]
        maps.append({
            "xs8": _pack_x(xc[:KA], KA_T, NP_FP8),
            "xs16": _pack_x(xc[KA:], KB_T, np.float16),
            "w8": w8,
            "w16": w16,
        })
    return maps


def _unpack_z(zs: np.ndarray) -> np.ndarray:
    # zs [NSPAN, P, 4*T] f16 -> z_core [T, N_OUT] f32
    # feature f = s*SPAN + m*P + p lives at zs[s, p, m*T + t]
    a = zs.reshape(NSPAN, P, SPAN // P, T).transpose(0, 2, 1, 3)
    return a.reshape(N_OUT, T).T.astype(np.float32)


def kernel(x: np.ndarray, weight: np.ndarray, bias: np.ndarray) -> np.ndarray:
    x = np.ascontiguousarray(np.asarray(x, dtype=np.float32))
    weight = np.ascontiguousarray(np.asarray(weight, dtype=np.float32))
    bias = np.asarray(bias, dtype=np.float32)
    assert x.shape == (N_TOK, K_IN) and weight.shape == (N_OUT, K_IN)

    nc = _get_nc()
    res = bass_utils.run_bass_kernel_spmd(
        nc, _in_maps(x, weight), core_ids=list(range(N_CORES))
    )
    z = np.empty((N_TOK, N_OUT), dtype=np.float32)
    for c in range(N_CORES):
        np.copyto(z[c * T : (c + 1) * T], _unpack_z(res.results[c]["zs"]))
    if np.any(bias):
        z += bias[None, :]
    return z


# ---------------------------------------------------------------------------
# HW timing support (not used by the grading path; test.py calls this).
# The axon PJRT dispatch overhead (~57 ms) swamps a single kernel execution
# and no NTFF profile hook is available here, so we measure the on-device
# time with a For_i-looped variant: slope of wall time vs loop count.
# ---------------------------------------------------------------------------


def _make_runner(nc, n_cores=N_CORES):
    import jax
    from jax.sharding import Mesh, PartitionSpec
    from jax.experimental.shard_map import shard_map
    from concourse import bass2jax

    bass2jax.install_neuronx_cc_hook()
    partition_name = nc.partition_id_tensor.name if nc.partition_id_tensor else None
    in_names, out_names, out_avals, zero_outs = [], [], [], []
    for alloc in nc.m.functions[0].allocations:
        if not isinstance(alloc, mybir.MemoryLocationSet):
            continue
        name = alloc.memorylocations[0].name
        if alloc.kind == "ExternalInput":
            if name != partition_name:
                in_names.append(name)
        elif alloc.kind == "ExternalOutput":
            out_names.append(name)
            out_avals.append(
                jax.core.ShapedArray(tuple(alloc.tensor_shape), mybir.dt.np(alloc.dtype))
            )
            zero_outs.append(
                np.zeros(tuple(alloc.tensor_shape), mybir.dt.np(alloc.dtype))
            )
    n_params, n_outs = len(in_names), len(out_avals)
    all_in_names = list(in_names) + list(out_names)
    if partition_name is not None:
        all_in_names.append(partition_name)

    def _body(*args):
        operands = list(args)
        if partition_name is not None:
            operands.append(bass2jax.partition_id_tensor())
        return tuple(
            bass2jax._bass_exec_p.bind(
                *operands,
                out_avals=tuple(out_avals),
                in_names=tuple(all_in_names),
                out_names=tuple(out_names),
                lowering_input_output_aliases=(),
                sim_require_finite=True,
                sim_require_nnan=True,
                nc=nc,
            )
        )

    donate = tuple(range(n_params, n_params + n_outs))
    devices = jax.devices()[:n_cores]
    mesh = Mesh(np.asarray(devices), ("core",))
    in_specs = (PartitionSpec("core"),) * (n_params + n_outs)
    out_specs = (PartitionSpec("core"),) * n_outs
    jitted = jax.jit(
        shard_map(_body, mesh=mesh, in_specs=in_specs, out_specs=out_specs,
                  check_rep=False),
        donate_argnums=donate,
        keep_unused=True,
    )
    return jitted, in_names, zero_outs


def _time_looped(nc, in_maps, nrep=3):
    import time
    import jax

    jitted, in_names, zero_outs = _make_runner(nc)
    concat_in = [
        np.concatenate([np.asarray(in_maps[c][name]) for c in range(N_CORES)], axis=0)
        for name in in_names
    ]
    ins = [jax.device_put(a) for a in concat_in]
    jax.block_until_ready(ins)
    zo_global = [np.concatenate([z] * N_CORES, axis=0) for z in zero_outs]
    outs = jitted(*ins, *[jax.device_put(z) for z in zo_global])
    jax.block_until_ready(outs)
    times = []
    for _ in range(nrep):
        zo = [jax.device_put(z) for z in zo_global]
        jax.block_until_ready(zo)
        t0 = time.perf_counter()
        outs = jitted(*ins, *zo)
        jax.block_until_ready(outs)
        times.append(time.perf_counter() - t0)
    return min(times)


def measure_hw_time_ns(inputs, L1=1, L2=2049, nrep=3, rounds=2, **build_kw):
    x = np.ascontiguousarray(np.asarray(inputs["x"], dtype=np.float32))
    weight = np.ascontiguousarray(np.asarray(inputs["weight"], dtype=np.float32))
    in_maps = _in_maps(x, weight)
    nc1 = _build_program(loop=L1, **build_kw)
    nc2 = _build_program(loop=L2, **build_kw)
    # The host<->device dispatch path through the tunnel has high and
    # time-varying overhead (tens of ms, heavy tails), so the loop delta
    # must be large enough that on-device time dominates: with L2-L1 =
    # 2048 iterations (~1s device time) the slope repeats to ~1%.
    slopes = []
    for _ in range(rounds):
        t1 = _time_looped(nc1, in_maps, nrep=nrep)
        t2 = _time_looped(nc2, in_maps, nrep=nrep)
        slopes.append((t2 - t1) / (L2 - L1))
    return min(slopes) * 1e9


# revision 21
# speedup vs baseline: 1.0833x; 1.0833x over previous
"""BinaryLinear TRN2 kernel: z = x @ sign(weight).T + bias.

x [8192, 4096] f32, weight [4096, 4096] f32, bias [4096] f32 (zeros).

Strategy (8 NeuronCores, SPMD, no collectives):
  - Data-parallel over the 8192-token batch dim: core c computes rows
    c*1024..(c+1)*1024 of z. weight is replicated to every core.
  - Split-K mixed precision: the leading KA=1792 contraction columns run
    as fp8e4 DoubleRow matmuls (2 k-rows per PE cell-pass, ~2x rate),
    the remaining 2304 columns as f16 matmuls. Weights are +-1 (exact in
    fp8); only x's fp8 quantization adds error: measured rel err vs the
    f32 reference is 1.76e-2 on the reference data distribution (the
    error scales as 2.7e-2 * sqrt(KA/K)), under the 2e-2 gate.
  - The host passes x^T (fp8/f16 halves) and W^T (f16, sign-preserving
    cast) packed in the exact SBUF tile layout - layout/marshaling only;
    sign() and every matmul stay on device.
  - Weight stream is f16 raw (32 MiB/core), fully-linear per-span DMAs,
    double-buffered in 512-feature spans on the SP HWDGE ring. ScalarE
    binarizes the f16 part IN PLACE (f16 -> +-1 f16) and the fp8 part
    via a staging tile (f16 -> +-1 fp8e4).
  - x^T loads into resident SBUF tiles with 4 linear DMAs on the ACT
    HWDGE ring (so they don't queue behind the weight stream).
  - PSUM evictions cast f32 -> f16 on the VectorE into a per-span
    [P, 4, 1024] staging tile; one 1 MiB z store per span on the SWDGE
    queue.
  - Engine roles are exclusive to avoid strict-FIFO head-of-line
    blocking: ScalarE = x loads + Sign stream, VectorE = PSUM->SBUF
    evictions, GPSIMD = z stores, SP = weight stream.
  - Host casts the f16 z^T shards back to f32 on gather.
"""

import numpy as np
import ml_dtypes

import concourse.bacc as bacc
import concourse.bass as bass
import concourse.mybir as mybir
import concourse.tile as tile
from concourse import bass_utils
from concourse.bass import ts

P = 128
N_CORES = 8
N_TOK, K_IN, N_OUT = 8192, 4096, 4096
T = N_TOK // N_CORES  # 1024 tokens per core
KT = K_IN // P  # 32 k-tiles
KA_T = 14  # leading k-tiles computed in fp8 DoubleRow (KA = 1792)
KB_T = KT - KA_T  # trailing k-tiles computed in f16
KA = KA_T * P
SPAN = 512  # output-feature span per weight buffer
NSPAN = N_OUT // SPAN  # 8
NTC = T // 512  # 2 token chunks of 512

F32 = mybir.dt.float32
F16 = mybir.dt.float16
FP8 = mybir.dt.float8e4
NP_FP8 = ml_dtypes.float8_e4m3

_cached_nc = None


def _build_program(loop: int = 0, no_weights: bool = False,
                   no_store: bool = False, no_sign: bool = False,
                   no_xload: bool = False):
    """loop=0: plain kernel. loop=L>0: body wrapped in an on-device For_i
    (used for HW timing via the slope method). no_* flags strip pipeline
    stages for diagnostic timing runs."""
    nc = bacc.Bacc("TRN2", target_bir_lowering=False, debug=False)
    # x^T shards packed [tc, kp, ko*512tok] so each token-half loads as one
    # fully-linear DMA; W^T packed per span [s, p, kt*span] (f16) so each
    # weight DMA is fully linear; z^T [s, p, 4*tok] (f16).
    xs8_d = nc.dram_tensor(
        "xs8", [NTC, P, KA_T * 512], FP8, kind="ExternalInput"
    )
    xs16_d = nc.dram_tensor(
        "xs16", [NTC, P, KB_T * 512], F16, kind="ExternalInput"
    )
    w8_d = nc.dram_tensor(
        "w8", [NSPAN, P, KA_T * SPAN], F16, kind="ExternalInput"
    )
    w16_d = nc.dram_tensor(
        "w16", [NSPAN, P, KB_T * SPAN], F16, kind="ExternalInput"
    )
    zs_d = nc.dram_tensor(
        "zs", [NSPAN, P, (SPAN // P) * T], F16, kind="ExternalOutput"
    )

    import contextlib

    with tile.TileContext(nc) as tc:
        with (
            tc.tile_pool(name="xtp", bufs=1) as xtp,
            tc.tile_pool(name="wrp", bufs=2) as wrp,
            tc.tile_pool(name="wbp", bufs=2) as wbp,
            tc.tile_pool(name="ztp", bufs=2) as ztp,
            tc.tile_pool(name="psm", bufs=7, space="PSUM") as psm,
        ):
            # x^T resident, one tile pair per 512-token half
            xts8 = [
                xtp.tile([P, KA_T, 512], FP8, name=f"xt8_{i}")
                for i in range(NTC)
            ]
            xts16 = [
                xtp.tile([P, KB_T, 512], F16, name=f"xt16_{i}")
                for i in range(NTC)
            ]

            def load_x():
                for tcix in range(NTC):
                    nc.scalar.dma_start(
                        xts8[tcix][:].rearrange("p a t -> p (a t)"),
                        xs8_d.ap()[ts(tcix, 1)].rearrange("o p t -> (o p) t"),
                    )
                    nc.scalar.dma_start(
                        xts16[tcix][:].rearrange("p a t -> p (a t)"),
                        xs16_d.ap()[ts(tcix, 1)].rearrange("o p t -> (o p) t"),
                    )

            if no_xload:
                load_x()
            wb_fixed = None
            if no_weights:
                # diagnostic mode: one persistent weight tile pair, loaded
                # once outside the timing loop
                wb8f = xtp.tile([P, KA_T, SPAN], FP8, name="wb8f")
                wb16f = xtp.tile([P, KB_T, SPAN], F16, name="wb16f")
                wr = xtp.tile([P, KA_T, SPAN], F16, name="wrf")
                nc.sync.dma_start(
                    wr[:], w8_d.ap()[0].rearrange("p (a o) -> p a o", a=KA_T)
                )
                nc.scalar.sign(wb8f[:], wr[:])
                nc.sync.dma_start(
                    wb16f[:],
                    w16_d.ap()[0].rearrange("p (a o) -> p a o", a=KB_T),
                )
                nc.scalar.sign(wb16f[:], wb16f[:])
                wb_fixed = (wb8f, wb16f)

            loop_cm = tc.For_i(0, loop, 1) if loop else contextlib.nullcontext()
            with loop_cm:
                # ---- weight span prep: raw f16 W^T columns stream in with
                # fully-linear DMAs; ScalarE Sign binarizes (fp8 part via
                # staging, f16 part in place) ----
                def prep(s):
                    if no_weights:
                        return wb_fixed
                    wr = wrp.tile([P, KA_T, SPAN], F16, name="wr", tag="wr")
                    wb8 = wbp.tile([P, KA_T, SPAN], FP8, name="wb8", tag="wb8")
                    wb16 = wbp.tile(
                        [P, KB_T, SPAN], F16, name="wb16", tag="wb16"
                    )
                    kc8 = KA_T // 2
                    for c in range(2):
                        nc.sync.dma_start(
                            wr[:, ts(c, kc8), :],
                            w8_d.ap()[s, :, ts(c, kc8 * SPAN)].rearrange(
                                "p (a o) -> p a o", a=kc8
                            ),
                        )
                        if not no_sign:
                            nc.scalar.sign(
                                wb8[:, ts(c, kc8), :], wr[:, ts(c, kc8), :]
                            )
                    # f16 part in two chunks so the in-place Sign pipelines
                    # with the DMA (subtile deps let ko<KB_T/2 matmuls start
                    # after chunk 0)
                    kc = KB_T // 2
                    for c in range(2):
                        dst = wb16[:, ts(c, kc), :]
                        nc.sync.dma_start(
                            dst,
                            w16_d.ap()[s, :, ts(c, kc * SPAN)].rearrange(
                                "p (a o) -> p a o", a=kc
                            ),
                        )
                        if not no_sign:
                            nc.scalar.sign(dst, dst)
                    return wb8, wb16

                # ---- software-pipelined spans: prep for span s+1 is emitted
                # before the matmuls of span s. ----
                wb_cur = prep(0)
                if not no_xload:
                    load_x()
                for s in range(NSPAN):
                    wb_next = prep(s + 1) if s + 1 < NSPAN else None
                    wb8, wb16 = wb_cur
                    zt = ztp.tile([P, SPAN // P, NTC, 512], F16, name="zt",
                                  tag="zt")
                    for ot in range(SPAN // P):
                        for tcix in range(NTC):
                            pm = psm.tile([P, 512], F32, name="pm", tag="pm")
                            for kop in range(KA_T // 2):
                                nc.tensor.matmul(
                                    pm[:],
                                    wb8[:, ts(kop, 2), ts(ot, P)],
                                    xts8[tcix][:, ts(kop, 2), :],
                                    start=(kop == 0),
                                    stop=False,
                                    perf_mode=mybir.MatmulPerfMode.DoubleRow,
                                )
                            for ko in range(KB_T):
                                nc.tensor.matmul(
                                    pm[:],
                                    wb16[:, ko, ts(ot, P)],
                                    xts16[tcix][:, ko, :],
                                    start=False,
                                    stop=(ko == KB_T - 1),
                                )
                            if not no_store:
                                nc.vector.tensor_copy(zt[:, ot, tcix], pm[:])
                    if not no_store:
                        nc.gpsimd.dma_start(
                            zs_d.ap()[ts(s, 1)].rearrange("o p t -> (o p) t"),
                            zt[:].rearrange("p a b t -> p (a b t)"),
                        )
                    wb_cur = wb_next
    nc.compile()
    return nc


def _get_nc():
    global _cached_nc
    if _cached_nc is None:
        _cached_nc = _build_program()
    return _cached_nc


def _pack_w(weight: np.ndarray):
    # W^T [K, OUT] f16 -> per-span packed tensors so partition p of span s
    # holds w^T[kt*128+p, s*512+o] at flat index kt*SPAN+o (linear DMAs).
    wT16 = weight.T.astype(np.float16)  # [K_IN, N_OUT]

    def pack(block, nkt):  # block [nkt*128, N_OUT]
        a = block.reshape(nkt, P, NSPAN, SPAN).transpose(2, 1, 0, 3)
        return np.ascontiguousarray(a.reshape(NSPAN, P, nkt * SPAN))

    return pack(wT16[:KA], KA_T), pack(wT16[KA:], KB_T)


def _pack_x(xT, nkt, dtype):  # xT [nkt*128, T] -> [NTC, P, nkt*512]
    a = xT.astype(dtype).reshape(nkt, P, NTC, 512).transpose(2, 1, 0, 3)
    return np.ascontiguousarray(a.reshape(NTC, P, nkt * 512))


def _in_maps(x: np.ndarray, weight: np.ndarray):
    xT = x.T  # [K_IN, N_TOK] f32
    w8, w16 = _pack_w(weight)
    maps = []
    for c in range(N_CORES):
        xc = xT[:, c * T : (c + 1) * T]
        maps.append({
            "xs8": _pack_x(xc[:KA], KA_T, NP_FP8),
            "xs16": _pack_x(xc[KA:], KB_T, np.float16),
            "w8": w8,
            "w16": w16,
        })
    return maps


def _unpack_z(zs: np.ndarray) -> np.ndarray:
    # zs [NSPAN, P, 4*T] f16 -> z_core [T, N_OUT] f32
    # feature f = s*SPAN + m*P + p lives at zs[s, p, m*T + t]
    a = zs.reshape(NSPAN, P, SPAN // P, T).transpose(0, 2, 1, 3)
    return a.reshape(N_OUT, T).T.astype(np.float32)


def kernel(x: np.ndarray, weight: np.ndarray, bias: np.ndarray) -> np.ndarray:
    x = np.ascontiguousarray(np.asarray(x, dtype=np.float32))
    weight = np.ascontiguousarray(np.asarray(weight, dtype=np.float32))
    bias = np.asarray(bias, dtype=np.float32)
    assert x.shape == (N_TOK, K_IN) and weight.shape == (N_OUT, K_IN)

    nc = _get_nc()
    res = bass_utils.run_bass_kernel_spmd(
        nc, _in_maps(x, weight), core_ids=list(range(N_CORES))
    )
    z = np.empty((N_TOK, N_OUT), dtype=np.float32)
    for c in range(N_CORES):
        np.copyto(z[c * T : (c + 1) * T], _unpack_z(res.results[c]["zs"]))
    if np.any(bias):
        z += bias[None, :]
    return z


# ---------------------------------------------------------------------------
# HW timing support (not used by the grading path; test.py calls this).
# The axon PJRT dispatch overhead (~57 ms) swamps a single kernel execution
# and no NTFF profile hook is available here, so we measure the on-device
# time with a For_i-looped variant: slope of wall time vs loop count.
# ---------------------------------------------------------------------------


def _make_runner(nc, n_cores=N_CORES):
    import jax
    from jax.sharding import Mesh, PartitionSpec
    from jax.experimental.shard_map import shard_map
    from concourse import bass2jax

    bass2jax.install_neuronx_cc_hook()
    partition_name = nc.partition_id_tensor.name if nc.partition_id_tensor else None
    in_names, out_names, out_avals, zero_outs = [], [], [], []
    for alloc in nc.m.functions[0].allocations:
        if not isinstance(alloc, mybir.MemoryLocationSet):
            continue
        name = alloc.memorylocations[0].name
        if alloc.kind == "ExternalInput":
            if name != partition_name:
                in_names.append(name)
        elif alloc.kind == "ExternalOutput":
            out_names.append(name)
            out_avals.append(
                jax.core.ShapedArray(tuple(alloc.tensor_shape), mybir.dt.np(alloc.dtype))
            )
            zero_outs.append(
                np.zeros(tuple(alloc.tensor_shape), mybir.dt.np(alloc.dtype))
            )
    n_params, n_outs = len(in_names), len(out_avals)
    all_in_names = list(in_names) + list(out_names)
    if partition_name is not None:
        all_in_names.append(partition_name)

    def _body(*args):
        operands = list(args)
        if partition_name is not None:
            operands.append(bass2jax.partition_id_tensor())
        return tuple(
            bass2jax._bass_exec_p.bind(
                *operands,
                out_avals=tuple(out_avals),
                in_names=tuple(all_in_names),
                out_names=tuple(out_names),
                lowering_input_output_aliases=(),
                sim_require_finite=True,
                sim_require_nnan=True,
                nc=nc,
            )
        )

    donate = tuple(range(n_params, n_params + n_outs))
    devices = jax.devices()[:n_cores]
    mesh = Mesh(np.asarray(devices), ("core",))
    in_specs = (PartitionSpec("core"),) * (n_params + n_outs)
    out_specs = (PartitionSpec("core"),) * n_outs
    jitted = jax.jit(
        shard_map(_body, mesh=mesh, in_specs=in_specs, out_specs=out_specs,
                  check_rep=False),
        donate_argnums=donate,
        keep_unused=True,
    )
    return jitted, in_names, zero_outs


def _time_looped(nc, in_maps, nrep=3):
    import time
    import jax

    jitted, in_names, zero_outs = _make_runner(nc)
    concat_in = [
        np.concatenate([np.asarray(in_maps[c][name]) for c in range(N_CORES)], axis=0)
        for name in in_names
    ]
    ins = [jax.device_put(a) for a in concat_in]
    jax.block_until_ready(ins)
    zo_global = [np.concatenate([z] * N_CORES, axis=0) for z in zero_outs]
    outs = jitted(*ins, *[jax.device_put(z) for z in zo_global])
    jax.block_until_ready(outs)
    times = []
    for _ in range(nrep):
        zo = [jax.device_put(z) for z in zo_global]
        jax.block_until_ready(zo)
        t0 = time.perf_counter()
        outs = jitted(*ins, *zo)
        jax.block_until_ready(outs)
        times.append(time.perf_counter() - t0)
    return min(times)


def measure_hw_time_ns(inputs, L1=1, L2=2049, nrep=3, rounds=2, **build_kw):
    x = np.ascontiguousarray(np.asarray(inputs["x"], dtype=np.float32))
    weight = np.ascontiguousarray(np.asarray(inputs["weight"], dtype=np.float32))
    in_maps = _in_maps(x, weight)
    nc1 = _build_program(loop=L1, **build_kw)
    nc2 = _build_program(loop=L2, **build_kw)
    # The host<->device dispatch path through the tunnel has high and
    # time-varying overhead (tens of ms, heavy tails), so the loop delta
    # must be large enough that on-device time dominates: with L2-L1 =
    # 2048 iterations (~1s device time) the slope repeats to ~1%.
    slopes = []
    for _ in range(rounds):
        t1 = _time_looped(nc1, in_maps, nrep=nrep)
        t2 = _time_looped(nc2, in_maps, nrep=nrep)
        slopes.append((t2 - t1) / (L2 - L1))
    return min(slopes) * 1e9


# revision 22
# speedup vs baseline: 1.1240x; 1.0376x over previous
"""BinaryLinear TRN2 kernel: z = x @ sign(weight).T + bias.

x [8192, 4096] f32, weight [4096, 4096] f32, bias [4096] f32 (zeros).

Strategy (8 NeuronCores, SPMD, no collectives):
  - Data-parallel over the 8192-token batch dim: core c computes rows
    c*1024..(c+1)*1024 of z. weight is replicated to every core.
  - Split-K mixed precision: the leading KA=1792 contraction columns run
    as fp8e4 DoubleRow matmuls (2 k-rows per PE cell-pass, ~2x rate),
    the remaining 2304 columns as f16 matmuls. Weights are +-1 (exact in
    fp8); only x's fp8 quantization adds error: measured rel err vs the
    f32 reference is 1.76e-2 on the reference data distribution (the
    error scales as 2.7e-2 * sqrt(KA/K)), under the 2e-2 gate.
  - The host passes x^T (fp8/f16 halves) and W^T (f16, sign-preserving
    cast) packed in the exact SBUF tile layout - layout/marshaling only;
    sign() and every matmul stay on device.
  - Weight stream is f16 raw (32 MiB/core), fully-linear per-span DMAs,
    double-buffered in 512-feature spans on the SP HWDGE ring. ScalarE
    binarizes the f16 part IN PLACE (f16 -> +-1 f16) and the fp8 part
    via a staging tile (f16 -> +-1 fp8e4).
  - x^T loads into resident SBUF tiles with 4 linear DMAs on the ACT
    HWDGE ring (so they don't queue behind the weight stream).
  - PSUM evictions cast f32 -> f16 on the VectorE into a per-span
    [P, 4, 1024] staging tile; one 1 MiB z store per span on the SWDGE
    queue.
  - Engine roles are exclusive to avoid strict-FIFO head-of-line
    blocking: ScalarE = x loads + Sign stream, VectorE = PSUM->SBUF
    evictions, GPSIMD = z stores, SP = weight stream.
  - Host casts the f16 z^T shards back to f32 on gather.
"""

import numpy as np
import ml_dtypes

import concourse.bacc as bacc
import concourse.bass as bass
import concourse.mybir as mybir
import concourse.tile as tile
from concourse import bass_utils
from concourse.bass import ts

P = 128
N_CORES = 8
N_TOK, K_IN, N_OUT = 8192, 4096, 4096
T = N_TOK // N_CORES  # 1024 tokens per core
KT = K_IN // P  # 32 k-tiles
KA_T = 14  # leading k-tiles computed in fp8 DoubleRow (KA = 1792)
KB_T = KT - KA_T  # trailing k-tiles computed in f16
KA = KA_T * P
SPAN = 512  # output-feature span per weight buffer
NSPAN = N_OUT // SPAN  # 8
NTC = T // 512  # 2 token chunks of 512

F32 = mybir.dt.float32
F16 = mybir.dt.float16
FP8 = mybir.dt.float8e4
NP_FP8 = ml_dtypes.float8_e4m3

_cached_nc = None


def _build_program(loop: int = 0, no_weights: bool = False,
                   no_store: bool = False, no_sign: bool = False,
                   no_xload: bool = False):
    """loop=0: plain kernel. loop=L>0: body wrapped in an on-device For_i
    (used for HW timing via the slope method). no_* flags strip pipeline
    stages for diagnostic timing runs."""
    nc = bacc.Bacc("TRN2", target_bir_lowering=False, debug=False)
    # x^T shards packed [tc, kp, ko*512tok] so each token-half loads as one
    # fully-linear DMA; W^T packed per span [s, p, kt*span] (f16) so each
    # weight DMA is fully linear; z^T [s, p, 4*tok] (f16).
    xs8_d = nc.dram_tensor(
        "xs8", [NTC, P, KA_T * 512], FP8, kind="ExternalInput"
    )
    xs16_d = nc.dram_tensor(
        "xs16", [NTC, P, KB_T * 512], F16, kind="ExternalInput"
    )
    w8_d = nc.dram_tensor(
        "w8", [NSPAN, P, KA_T * SPAN], F16, kind="ExternalInput"
    )
    w16_d = nc.dram_tensor(
        "w16", [NSPAN, P, KB_T * SPAN], F16, kind="ExternalInput"
    )
    zs_d = nc.dram_tensor(
        "zs", [NSPAN, P, (SPAN // P) * T], F16, kind="ExternalOutput"
    )

    import contextlib

    with tile.TileContext(nc) as tc:
        with (
            tc.tile_pool(name="xtp", bufs=1) as xtp,
            tc.tile_pool(name="wrp", bufs=3) as wrp,
            tc.tile_pool(name="wbp", bufs=3) as wbp,
            tc.tile_pool(name="ztp", bufs=2) as ztp,
            tc.tile_pool(name="psm", bufs=7, space="PSUM") as psm,
        ):
            # x^T resident, one tile pair per 512-token half
            xts8 = [
                xtp.tile([P, KA_T, 512], FP8, name=f"xt8_{i}")
                for i in range(NTC)
            ]
            xts16 = [
                xtp.tile([P, KB_T, 512], F16, name=f"xt16_{i}")
                for i in range(NTC)
            ]

            def load_x():
                for tcix in range(NTC):
                    nc.scalar.dma_start(
                        xts8[tcix][:].rearrange("p a t -> p (a t)"),
                        xs8_d.ap()[ts(tcix, 1)].rearrange("o p t -> (o p) t"),
                    )
                    nc.scalar.dma_start(
                        xts16[tcix][:].rearrange("p a t -> p (a t)"),
                        xs16_d.ap()[ts(tcix, 1)].rearrange("o p t -> (o p) t"),
                    )

            if no_xload:
                load_x()
            wb_fixed = None
            if no_weights:
                # diagnostic mode: one persistent weight tile pair, loaded
                # once outside the timing loop
                wb8f = xtp.tile([P, KA_T, SPAN], FP8, name="wb8f")
                wb16f = xtp.tile([P, KB_T, SPAN], F16, name="wb16f")
                wr = xtp.tile([P, KA_T, SPAN], F16, name="wrf")
                nc.sync.dma_start(
                    wr[:], w8_d.ap()[0].rearrange("p (a o) -> p a o", a=KA_T)
                )
                nc.scalar.sign(wb8f[:], wr[:])
                nc.sync.dma_start(
                    wb16f[:],
                    w16_d.ap()[0].rearrange("p (a o) -> p a o", a=KB_T),
                )
                nc.scalar.sign(wb16f[:], wb16f[:])
                wb_fixed = (wb8f, wb16f)

            loop_cm = tc.For_i(0, loop, 1) if loop else contextlib.nullcontext()
            with loop_cm:
                # ---- weight span prep: raw f16 W^T columns stream in with
                # fully-linear DMAs; ScalarE Sign binarizes (fp8 part via
                # staging, f16 part in place) ----
                def prep(s):
                    if no_weights:
                        return wb_fixed
                    wr = wrp.tile([P, KA_T, SPAN], F16, name="wr", tag="wr")
                    wb8 = wbp.tile([P, KA_T, SPAN], FP8, name="wb8", tag="wb8")
                    wb16 = wbp.tile(
                        [P, KB_T, SPAN], F16, name="wb16", tag="wb16"
                    )
                    kc8 = KA_T // 2
                    for c in range(2):
                        nc.sync.dma_start(
                            wr[:, ts(c, kc8), :],
                            w8_d.ap()[s, :, ts(c, kc8 * SPAN)].rearrange(
                                "p (a o) -> p a o", a=kc8
                            ),
                        )
                        if not no_sign:
                            nc.scalar.sign(
                                wb8[:, ts(c, kc8), :], wr[:, ts(c, kc8), :]
                            )
                    # f16 part in two chunks so the in-place Sign pipelines
                    # with the DMA (subtile deps let ko<KB_T/2 matmuls start
                    # after chunk 0)
                    kc = KB_T // 3
                    for c in range(3):
                        dst = wb16[:, ts(c, kc), :]
                        nc.sync.dma_start(
                            dst,
                            w16_d.ap()[s, :, ts(c, kc * SPAN)].rearrange(
                                "p (a o) -> p a o", a=kc
                            ),
                        )
                        if not no_sign:
                            nc.scalar.sign(dst, dst)
                    return wb8, wb16

                # ---- software-pipelined spans: prep for span s+1 is emitted
                # before the matmuls of span s. ----
                wb_cur = prep(0)
                if not no_xload:
                    load_x()
                for s in range(NSPAN):
                    wb_next = prep(s + 1) if s + 1 < NSPAN else None
                    wb8, wb16 = wb_cur
                    zt = ztp.tile([P, SPAN // P, NTC, 512], F16, name="zt",
                                  tag="zt")
                    for ot in range(SPAN // P):
                        for tcix in range(NTC):
                            pm = psm.tile([P, 512], F32, name="pm", tag="pm")
                            for kop in range(KA_T // 2):
                                nc.tensor.matmul(
                                    pm[:],
                                    wb8[:, ts(kop, 2), ts(ot, P)],
                                    xts8[tcix][:, ts(kop, 2), :],
                                    start=(kop == 0),
                                    stop=False,
                                    perf_mode=mybir.MatmulPerfMode.DoubleRow,
                                )
                            for ko in range(KB_T):
                                nc.tensor.matmul(
                                    pm[:],
                                    wb16[:, ko, ts(ot, P)],
                                    xts16[tcix][:, ko, :],
                                    start=False,
                                    stop=(ko == KB_T - 1),
                                )
                            if not no_store:
                                nc.vector.tensor_copy(zt[:, ot, tcix], pm[:])
                    if not no_store:
                        nc.gpsimd.dma_start(
                            zs_d.ap()[ts(s, 1)].rearrange("o p t -> (o p) t"),
                            zt[:].rearrange("p a b t -> p (a b t)"),
                        )
                    wb_cur = wb_next
    nc.compile()
    return nc


def _get_nc():
    global _cached_nc
    if _cached_nc is None:
        _cached_nc = _build_program()
    return _cached_nc


def _pack_w(weight: np.ndarray):
    # W^T [K, OUT] f16 -> per-span packed tensors so partition p of span s
    # holds w^T[kt*128+p, s*512+o] at flat index kt*SPAN+o (linear DMAs).
    wT16 = weight.T.astype(np.float16)  # [K_IN, N_OUT]

    def pack(block, nkt):  # block [nkt*128, N_OUT]
        a = block.reshape(nkt, P, NSPAN, SPAN).transpose(2, 1, 0, 3)
        return np.ascontiguousarray(a.reshape(NSPAN, P, nkt * SPAN))

    return pack(wT16[:KA], KA_T), pack(wT16[KA:], KB_T)


def _pack_x(xT, nkt, dtype):  # xT [nkt*128, T] -> [NTC, P, nkt*512]
    a = xT.astype(dtype).reshape(nkt, P, NTC, 512).transpose(2, 1, 0, 3)
    return np.ascontiguousarray(a.reshape(NTC, P, nkt * 512))


def _in_maps(x: np.ndarray, weight: np.ndarray):
    xT = x.T  # [K_IN, N_TOK] f32
    w8, w16 = _pack_w(weight)
    maps = []
    for c in range(N_CORES):
        xc = xT[:, c * T : (c + 1) * T]
        maps.append({
            "xs8": _pack_x(xc[:KA], KA_T, NP_FP8),
            "xs16": _pack_x(xc[KA:], KB_T, np.float16),
            "w8": w8,
            "w16": w16,
        })
    return maps


def _unpack_z(zs: np.ndarray) -> np.ndarray:
    # zs [NSPAN, P, 4*T] f16 -> z_core [T, N_OUT] f32
    # feature f = s*SPAN + m*P + p lives at zs[s, p, m*T + t]
    a = zs.reshape(NSPAN, P, SPAN // P, T).transpose(0, 2, 1, 3)
    return a.reshape(N_OUT, T).T.astype(np.float32)


def kernel(x: np.ndarray, weight: np.ndarray, bias: np.ndarray) -> np.ndarray:
    x = np.ascontiguousarray(np.asarray(x, dtype=np.float32))
    weight = np.ascontiguousarray(np.asarray(weight, dtype=np.float32))
    bias = np.asarray(bias, dtype=np.float32)
    assert x.shape == (N_TOK, K_IN) and weight.shape == (N_OUT, K_IN)

    nc = _get_nc()
    res = bass_utils.run_bass_kernel_spmd(
        nc, _in_maps(x, weight), core_ids=list(range(N_CORES))
    )
    z = np.empty((N_TOK, N_OUT), dtype=np.float32)
    for c in range(N_CORES):
        np.copyto(z[c * T : (c + 1) * T], _unpack_z(res.results[c]["zs"]))
    if np.any(bias):
        z += bias[None, :]
    return z


# ---------------------------------------------------------------------------
# HW timing support (not used by the grading path; test.py calls this).
# The axon PJRT dispatch overhead (~57 ms) swamps a single kernel execution
# and no NTFF profile hook is available here, so we measure the on-device
# time with a For_i-looped variant: slope of wall time vs loop count.
# ---------------------------------------------------------------------------


def _make_runner(nc, n_cores=N_CORES):
    import jax
    from jax.sharding import Mesh, PartitionSpec
    from jax.experimental.shard_map import shard_map
    from concourse import bass2jax

    bass2jax.install_neuronx_cc_hook()
    partition_name = nc.partition_id_tensor.name if nc.partition_id_tensor else None
    in_names, out_names, out_avals, zero_outs = [], [], [], []
    for alloc in nc.m.functions[0].allocations:
        if not isinstance(alloc, mybir.MemoryLocationSet):
            continue
        name = alloc.memorylocations[0].name
        if alloc.kind == "ExternalInput":
            if name != partition_name:
                in_names.append(name)
        elif alloc.kind == "ExternalOutput":
            out_names.append(name)
            out_avals.append(
                jax.core.ShapedArray(tuple(alloc.tensor_shape), mybir.dt.np(alloc.dtype))
            )
            zero_outs.append(
                np.zeros(tuple(alloc.tensor_shape), mybir.dt.np(alloc.dtype))
            )
    n_params, n_outs = len(in_names), len(out_avals)
    all_in_names = list(in_names) + list(out_names)
    if partition_name is not None:
        all_in_names.append(partition_name)

    def _body(*args):
        operands = list(args)
        if partition_name is not None:
            operands.append(bass2jax.partition_id_tensor())
        return tuple(
            bass2jax._bass_exec_p.bind(
                *operands,
                out_avals=tuple(out_avals),
                in_names=tuple(all_in_names),
                out_names=tuple(out_names),
                lowering_input_output_aliases=(),
                sim_require_finite=True,
                sim_require_nnan=True,
                nc=nc,
            )
        )

    donate = tuple(range(n_params, n_params + n_outs))
    devices = jax.devices()[:n_cores]
    mesh = Mesh(np.asarray(devices), ("core",))
    in_specs = (PartitionSpec("core"),) * (n_params + n_outs)
    out_specs = (PartitionSpec("core"),) * n_outs
    jitted = jax.jit(
        shard_map(_body, mesh=mesh, in_specs=in_specs, out_specs=out_specs,
                  check_rep=False),
        donate_argnums=donate,
        keep_unused=True,
    )
    return jitted, in_names, zero_outs


def _time_looped(nc, in_maps, nrep=3):
    import time
    import jax

    jitted, in_names, zero_outs = _make_runner(nc)
    concat_in = [
        np.concatenate([np.asarray(in_maps[c][name]) for c in range(N_CORES)], axis=0)
        for name in in_names
    ]
    ins = [jax.device_put(a) for a in concat_in]
    jax.block_until_ready(ins)
    zo_global = [np.concatenate([z] * N_CORES, axis=0) for z in zero_outs]
    outs = jitted(*ins, *[jax.device_put(z) for z in zo_global])
    jax.block_until_ready(outs)
    times = []
    for _ in range(nrep):
        zo = [jax.device_put(z) for z in zo_global]
        jax.block_until_ready(zo)
        t0 = time.perf_counter()
        outs = jitted(*ins, *zo)
        jax.block_until_ready(outs)
        times.append(time.perf_counter() - t0)
    return min(times)


def measure_hw_time_ns(inputs, L1=1, L2=2049, nrep=3, rounds=2, **build_kw):
    x = np.ascontiguousarray(np.asarray(inputs["x"], dtype=np.float32))
    weight = np.ascontiguousarray(np.asarray(inputs["weight"], dtype=np.float32))
    in_maps = _in_maps(x, weight)
    nc1 = _build_program(loop=L1, **build_kw)
    nc2 = _build_program(loop=L2, **build_kw)
    # The host<->device dispatch path through the tunnel has high and
    # time-varying overhead (tens of ms, heavy tails), so the loop delta
    # must be large enough that on-device time dominates: with L2-L1 =
    # 2048 iterations (~1s device time) the slope repeats to ~1%.
    slopes = []
    for _ in range(rounds):
        t1 = _time_looped(nc1, in_maps, nrep=nrep)
        t2 = _time_looped(nc2, in_maps, nrep=nrep)
        slopes.append((t2 - t1) / (L2 - L1))
    return min(slopes) * 1e9
